# revision 10
# baseline (speedup 1.0000x reference)
"""MLA-style latent attention kernel for Trainium2, 8-core SPMD. v4.

Problem: B=4, S=2048, H=2048, NH=16, HD=64, KVC=512, causal softmax attention.

Sharding: core i handles batch b = i//2 and head-half hp = i%2 (8 heads).
Each core computes its partial c_proj output (contraction over its 512 of the
1024 attn-out dims); the host sums the two partials per batch.

v4 design (v2 was 401us, v3's pre-exp DVE mask regressed to 467us):
  - WAVEFRONT: only hidden-chunk 0 of the q/latent projections (phase A)
    and kT/v of chunk 0 (phase A2) run standalone.  Chunks 1-3 of A/A2 and
    all of c_proj are emitted as background PE work inside the attention
    tile loop (2 pops per tile), so the ACT-bound exp stream and the
    PE-bound projection work overlap instead of running serially.
  - The score->exp edge is kept engine-pure (PE->ACT): the causal mask is
    a post-exp multiply on the otherwise-idle GPSIMD engine (v2 had it on
    DVE where evictions convoyed ahead of it; v3's pre-exp DVE add was
    worse).  DVE keeps only PSUM work (evictions, epilogue).
  - v1 head slots are [1 | v(64)]: the AV matmul's denominator row lands
    on PSUM partition 0, so reciprocal_approx_fast reads it directly (its
    nonzero-base-partition bug avoided) - epilogue is recip + gpsimd
    broadcast + one DVE normalize-mul off the PSUM accumulator.
  - All SBUF operands bf16 (host pre-transposes hidden and packs
    partition-major: every DMA line is >=4KB contiguous); PSUM f32;
    output written as bf16 partials, host sums in f32.
"""

import os
import sys

import numpy as np

for _p in ("/opt/trn_rl_repo",):
    if os.path.isdir(_p) and _p not in sys.path:
        sys.path.append(_p)

import concourse.bass as bass  # noqa: E402
import concourse.mybir as mybir  # noqa: E402
from concourse import bacc, tile  # noqa: E402
from concourse.bass_utils import run_bass_kernel_spmd  # noqa: E402

F32 = mybir.dt.float32
BF16 = mybir.dt.bfloat16
NPBF16 = mybir.dt.np(BF16)

B, S, H = 4, 2048, 2048
NH, HD = 16, 64
KVC = 512
DL = 512          # local q/k/v dims per core (8 heads x 64)
NHL = 8           # local heads
P = 128
SCALE = 0.125

HT = H // P       # 16 h-tiles
DT = DL // P      # 4 d-tiles (also head-pair index g)
CT = KVC // P     # 4 c-tiles
ST = S // P       # 16 s-tiles
SC = 512          # hidden s-chunk
NCH = S // SC     # 4 chunks

_CACHE = {}


def build_program():
    """Build + compile the per-core Bass program. Returns the Bacc module."""
    nc = bacc.Bacc("TRN2", target_bir_lowering=False, debug=False,
                   num_devices=8)

    hsx = nc.dram_tensor("hsx", [P, NCH, HT, SC], BF16,
                         kind="ExternalInput").ap()
    wq = nc.dram_tensor("wq", [P, HT, DL], BF16, kind="ExternalInput").ap()
    wkv = nc.dram_tensor("wkv", [P, HT, KVC], BF16,
                         kind="ExternalInput").ap()
    wk = nc.dram_tensor("wk", [P, CT, DL], BF16, kind="ExternalInput").ap()
    wv = nc.dram_tensor("wv", [P, CT, DL], BF16, kind="ExternalInput").ap()
    wo = nc.dram_tensor("wo", [P, DT, H], BF16, kind="ExternalInput").ap()
    out = nc.dram_tensor("out", [ST, P, H], BF16, kind="ExternalOutput").ap()

    from contextlib import ExitStack

    with tile.TileContext(nc) as tc, ExitStack() as stack:
        consts = stack.enter_context(tc.tile_pool(name="consts", bufs=1))
        # bigmask[j, z] = 1.0 if z - j >= 384 else 0.  Multiplies the
        # 128-wide diagonal band of an exp'd score tile (causal mask).
        bigmask = consts.tile([P, 896], BF16)
        nc.gpsimd.memset(bigmask, 1.0)
        nc.gpsimd.affine_select(
            out=bigmask, in_=bigmask,
            compare_op=mybir.AluOpType.is_ge, fill=0.0,
            base=-384, pattern=[[1, 896]], channel_multiplier=-1,
        )
        # Touch exp once so the ACT table set loads during the prologue.
        dummy = consts.tile([1, 32], F32)
        nc.vector.memset(dummy, 0.0)
        nc.scalar.activation(dummy, dummy,
                             mybir.ActivationFunctionType.Exp, scale=SCALE)

        persistA = stack.enter_context(tc.tile_pool(name="persistA", bufs=1))
        qT = persistA.tile([P, DT, S], BF16, tag="qT")
        latT = persistA.tile([P, CT, S], BF16, tag="latT")
        kT = persistA.tile([P, DT, S], BF16, tag="kT")
        v1 = persistA.tile([P, ST, NHL * (HD + 1)], BF16, tag="v1")
        # OT aliases latT s-block ib (latT's reads all precede the writes).
        OT = [latT[:, :, ib * 512:(ib + 1) * 512] for ib in range(4)]

        wB = stack.enter_context(tc.tile_pool(name="wB", bufs=1))
        wk_sb = wB.tile([P, CT, DL], BF16, tag="wk")
        nc.gpsimd.dma_start(wk_sb, wk)
        wv_sb = wB.tile([P, CT, DL], BF16, tag="wv")
        nc.gpsimd.dma_start(wv_sb, wv)
        wo_sb = wB.tile([P, DT, H], BF16, tag="wo")
        nc.gpsimd.dma_start(wo_sb, wo)

        wA = stack.enter_context(tc.tile_pool(name="wA", bufs=1))
        hinp = stack.enter_context(tc.tile_pool(name="hin", bufs=2))
        wq_sb = wA.tile([P, HT, DL], BF16, tag="wq")
        nc.sync.dma_start(wq_sb, wq)
        wkv_sb = wA.tile([P, HT, KVC], BF16, tag="wkv")
        hin = {}
        hin[0] = hinp.tile([P, HT, SC], BF16, tag="hin", name="hin0")
        nc.sync.dma_start(hin[0], hsx[:, 0])
        nc.sync.dma_start(wkv_sb, wkv)
        hin[1] = hinp.tile([P, HT, SC], BF16, tag="hin", name="hin1")
        nc.sync.dma_start(hin[1], hsx[:, 1])

        def emit_qlat_chain(c, dt_, w_sb, dst, ps):
            """One [128,512] projection chain: 16 MMs + bf16 eviction.
            Returns list of emission thunks (4 MMs each + evict)."""
            items = []
            for hq in range(4):
                def mms(hq=hq, ps=ps, c=c, dt_=dt_, w_sb=w_sb):
                    for hi in range(4):
                        ht = hq * 4 + hi
                        nc.tensor.matmul(
                            ps, w_sb[:, ht, dt_ * P:(dt_ + 1) * P],
                            hin[c][:, ht, :],
                            start=(ht == 0), stop=(ht == HT - 1))
                items.append(mms)
            def evict(ps=ps, c=c, dt_=dt_, dst=dst):
                nc.vector.tensor_copy(
                    dst[:, dt_, c * SC:(c + 1) * SC], ps)
            items.append(evict)
            return items

        def emit_kt_chain(sc, dt_, ps):
            items = []
            def mms(sc=sc, dt_=dt_, ps=ps):
                for ct in range(CT):
                    nc.tensor.matmul(
                        ps, wk_sb[:, ct, dt_ * P:(dt_ + 1) * P],
                        latT[:, ct, sc * 512:(sc + 1) * 512],
                        start=(ct == 0), stop=(ct == CT - 1))
            items.append(mms)
            def evict(sc=sc, dt_=dt_, ps=ps):
                nc.vector.tensor_copy(
                    kT[:, dt_, sc * 512:(sc + 1) * 512], ps)
            items.append(evict)
            return items

        def emit_v_chain(st, ps):
            items = []
            def mms(st=st, ps=ps):
                for ct in range(CT):
                    nc.tensor.matmul(
                        ps, latT[:, ct, st * P:(st + 1) * P],
                        wv_sb[:, ct, :],
                        start=(ct == 0), stop=(ct == CT - 1))
            items.append(mms)
            def evict(st=st, ps=ps):
                nc.vector.tensor_copy(
                    v1[:, st, :].rearrange(
                        "p (h e) -> p h e", e=HD + 1)[:, :, :HD],
                    ps.rearrange("p (h e) -> p h e", e=HD))
            items.append(evict)
            return items

        # ---------------- prologue: chunk 0 of A and A2 ----------------
        with tc.tile_pool(name="psA", bufs=2, space="PSUM") as psA:
            for dt_ in range(DT):
                ps = psA.tile([P, SC], F32, tag="psq")
                for it in emit_qlat_chain(0, dt_, wq_sb, qT, ps):
                    it()
                ps2 = psA.tile([P, SC], F32, tag="pslat")
                for it in emit_qlat_chain(0, dt_, wkv_sb, latT, ps2):
                    it()
        with tc.tile_pool(name="onep", bufs=1) as onep, \
             tc.tile_pool(name="psA2", bufs=2, space="PSUM") as psA2:
            # ones column LAST in each head's 65-wide slot (partition base
            # of engine ops must be 32-aligned, so den lands at 64)
            ones_src = onep.tile([P, ST, NHL], F32, tag="ones_src")
            nc.vector.memset(ones_src, 1.0)
            nc.vector.tensor_copy(
                v1.rearrange("p s (h e) -> p s h e", e=HD + 1)[:, :, :, HD],
                ones_src)
            for dt_ in range(DT):
                ps = psA2.tile([P, 512], F32, tag="psk")
                for it in emit_kt_chain(0, dt_, ps):
                    it()
            for st in range(4):
                ps = psA2.tile([P, 512], F32, tag="psv")
                for it in emit_v_chain(st, ps):
                    it()

        # ------- main phase: attention with background A/A2/c_proj -------
        def issue_scores(g, ib, jt, ps_sp):
            toff = (jt - 4 * ib) * P if jt >= 4 * ib else 0
            jts = slice(jt * P, (jt + 1) * P)
            iw = slice(ib * 512 + toff, (ib + 1) * 512)
            ps2 = ps_sp.tile([P, 2, 512], F32, tag="ps2")
            nc.tensor.matmul(
                ps2[:, 0, toff:], kT[0:HD, g, jts],
                qT[0:HD, g, iw], start=True, stop=True)
            nc.tensor.matmul(
                ps2[:, 1, toff:], kT[HD:P, g, jts],
                qT[HD:P, g, iw], start=True, stop=True)
            return ps2

        def attn_block(g, ib, ps_sp, ps_op, ptp, smallp,
                       first_ps2, next_block, bg_pop):
            se = (2 * g) * (HD + 1)
            so = (2 * g + 1) * (HD + 1)
            jt_max = 4 * (ib + 1)
            po_e = ps_op.tile([HD + 1, 512], F32, tag="po_e")
            po_o = ps_op.tile([HD + 1, 512], F32, tag="po_o")

            ps2_next = first_ps2 if first_ps2 is not None \
                else issue_scores(g, ib, 0, ps_sp)
            handoff = None
            for jt in range(jt_max):
                ps2 = ps2_next
                if jt + 1 < jt_max:
                    ps2_next = issue_scores(g, ib, jt + 1, ps_sp)
                elif next_block is not None:
                    handoff = issue_scores(*next_block, 0, ps_sp)
                bg_pop()
                toff = (jt - 4 * ib) * P if jt >= 4 * ib else 0
                pt2 = ptp.tile([P, 2, 512], BF16, tag="pt2")
                nc.scalar.activation(
                    pt2[:, :, toff:], ps2[:, :, toff:],
                    mybir.ActivationFunctionType.Exp, scale=SCALE)
                if jt >= 4 * ib:
                    # causal mask on GPSIMD: keeps the DVE eviction convoy
                    # off the exp->AV edge
                    mw = min(P, 512 - toff)
                    nc.gpsimd.tensor_mul(
                        out=pt2[:, :, toff:toff + mw],
                        in0=pt2[:, :, toff:toff + mw],
                        in1=bigmask[:, 384:384 + mw]
                        .rearrange("p (o f) -> p o f", o=1)
                        .broadcast_to((P, 2, mw)))
                last = jt == jt_max - 1
                nc.tensor.matmul(
                    po_e[:, toff:], v1[:, jt, se:se + HD + 1],
                    pt2[:, 0, toff:], start=(jt == 0), stop=last)
                nc.tensor.matmul(
                    po_o[:, toff:], v1[:, jt, so:so + HD + 1],
                    pt2[:, 1, toff:], start=(jt == 0), stop=last)
                bg_pop()
            for po, pbase in ((po_e, 0), (po_o, HD)):
                # po rows: [O^T(64) | den at 64].  Stage den at partition 0
                # (reciprocal_approx_fast mis-reads at nonzero base).
                dn = smallp.tile([1, 512], F32, tag="dn")
                nc.vector.tensor_copy(dn, po[HD:HD + 1, :])
                rc = smallp.tile([1, 512], F32, tag="rc")
                nc.vector.reciprocal_approx_fast(out=rc, in_=dn)
                rc64 = smallp.tile([HD, 512], F32, tag="rc64")
                nc.gpsimd.partition_broadcast(rc64, rc)
                nc.vector.tensor_mul(
                    out=OT[ib][pbase:pbase + HD, g, :],
                    in0=po[:HD, :], in1=rc64)
            return handoff

        def cproj_items(m, osbp, bgp):
            items = []
            osb = osbp.tile([P, H], BF16, tag="osb", name=f"osb{m}")
            ot = OT[m // 4]
            lm = m % 4
            for n in range(4):
                pc = bgp.tile([P, 512], F32, tag="bg", name=f"pc{m}_{n}")
                def mms(n=n, pc=pc, ot=ot, lm=lm):
                    for kt in range(DT):
                        nc.tensor.matmul(
                            pc, ot[:, kt, lm * P:(lm + 1) * P],
                            wo_sb[:, kt, n * 512:(n + 1) * 512],
                            start=(kt == 0), stop=(kt == DT - 1))
                items.append(mms)
                def evict(n=n, pc=pc, osb=osb, m=m):
                    nc.vector.tensor_copy(
                        osb[:, n * 512:(n + 1) * 512], pc)
                    if n == 3:
                        nc.sync.dma_start(out[m], osb)
                items.append(evict)
            return items

        with tc.tile_pool(name="ptp", bufs=6) as ptp, \
             tc.tile_pool(name="smallp", bufs=2) as smallp, \
             tc.tile_pool(name="osb_p", bufs=3) as osbp, \
             tc.tile_pool(name="ps_s", bufs=2, space="PSUM") as ps_sp, \
             tc.tile_pool(name="ps_o", bufs=1, space="PSUM") as ps_op, \
             tc.tile_pool(name="bgp", bufs=2, space="PSUM") as bgp:
            bg = []

            def bg_pop():
                if bg:
                    bg.pop(0)()

            def bg_flush():
                while bg:
                    bg.pop(0)()

            def queue_chunk(c):
                # background A (qT/latT) + A2 (kT/v1) for hidden chunk c
                for dt_ in range(DT):
                    ps = bgp.tile([P, SC], F32, tag="bg", name=f"q{c}{dt_}")
                    bg.extend(emit_qlat_chain(c, dt_, wq_sb, qT, ps))
                    ps = bgp.tile([P, SC], F32, tag="bg", name=f"l{c}{dt_}")
                    bg.extend(emit_qlat_chain(c, dt_, wkv_sb, latT, ps))
                for dt_ in range(DT):
                    ps = bgp.tile([P, 512], F32, tag="bg", name=f"k{c}{dt_}")
                    bg.extend(emit_kt_chain(c, dt_, ps))
                for s2 in range(4):
                    st = c * 4 + s2
                    ps = bgp.tile([P, 512], F32, tag="bg", name=f"v{st}")
                    bg.extend(emit_v_chain(st, ps))

            blocks = [(g, ib) for ib in range(4) for g in range(DT)]
            carry = None
            for k, (g, ib) in enumerate(blocks):
                if g == 0:
                    if ib + 1 < NCH:
                        queue_chunk(ib + 1)
                        if ib + 2 < NCH:
                            hin[ib + 2] = hinp.tile(
                                [P, HT, SC], BF16, tag="hin",
                                name=f"hin{ib + 2}")
                            nc.sync.dma_start(hin[ib + 2], hsx[:, ib + 2])
                    if ib >= 1:
                        for m in range(4 * (ib - 1), 4 * ib):
                            bg.extend(cproj_items(m, osbp, bgp))
                nxt = blocks[k + 1] if k + 1 < len(blocks) else None
                carry = attn_block(g, ib, ps_sp, ps_op, ptp, smallp,
                                   carry, nxt, bg_pop)
                if g == DT - 1:
                    bg_flush()
            for m in range(12, ST):
                for it in cproj_items(m, osbp, bgp):
                    it()

    nc.compile()
    return nc


def _get_program():
    if "nc" not in _CACHE:
        _CACHE["nc"] = build_program()
    return _CACHE["nc"]


def make_in_maps(hidden_states, wq, w_kv_down, w_k_up, w_v_up, w_out):
    """Host-side sharding: core i -> (batch i//2, head-half i%2).

    All operands are converted to bf16 and packed partition-major so every
    DMA line is contiguous:
      hsx[p, c, ht, s] = hidden[b][c*512+s, ht*128+p]   (pre-transposed)
      w*[p, t, d] with the contraction dim split as (t, p)
    """
    hsx_b = []
    for b in range(B):
        hb = np.asarray(hidden_states[b], dtype=np.float32).astype(NPBF16)
        x = hb.T.reshape(HT, P, NCH, SC).transpose(1, 2, 0, 3).copy()
        hsx_b.append(x)
    wq = np.asarray(wq, dtype=np.float32)
    w_kv_down = np.asarray(w_kv_down, dtype=np.float32)
    w_k_up = np.asarray(w_k_up, dtype=np.float32)
    w_v_up = np.asarray(w_v_up, dtype=np.float32)
    w_out = np.asarray(w_out, dtype=np.float32)
    wkv_p = w_kv_down.astype(NPBF16).reshape(HT, P, KVC).transpose(1, 0, 2) \
        .copy()
    in_maps = []
    for i in range(8):
        b, hp = i // 2, i % 2
        sl = slice(hp * DL, (hp + 1) * DL)
        in_maps.append({
            "hsx": hsx_b[b],
            "wq": wq[:, sl].astype(NPBF16).reshape(HT, P, DL)
            .transpose(1, 0, 2).copy(),
            "wkv": wkv_p,
            "wk": w_k_up[:, sl].astype(NPBF16).reshape(CT, P, DL)
            .transpose(1, 0, 2).copy(),
            "wv": w_v_up[:, sl].astype(NPBF16).reshape(CT, P, DL)
            .transpose(1, 0, 2).copy(),
            "wo": w_out[sl, :].astype(NPBF16).reshape(DT, P, H)
            .transpose(1, 0, 2).copy(),
        })
    return in_maps


def gather(results):
    """Host-side unshard: sum the two head-half bf16 partials per batch."""
    out = np.empty((B, S, H), dtype=np.float32)
    for b in range(B):
        p0 = np.asarray(results[2 * b]["out"]).astype(np.float32)
        p1 = np.asarray(results[2 * b + 1]["out"]).astype(np.float32)
        out[b] = (p0 + p1).reshape(S, H)
    return out


def kernel(hidden_states, wq, w_kv_down, w_k_up, w_v_up, w_out, _trace=False):
    nc = _get_program()
    in_maps = make_in_maps(hidden_states, wq, w_kv_down, w_k_up, w_v_up,
                           w_out)
    res = run_bass_kernel_spmd(nc, in_maps, list(range(8)), trace=_trace)
    out = gather(res.results)
    if _trace:
        return out, res
    return out


# revision 12
# speedup vs baseline: 1.0220x; 1.0220x over previous
"""MLA-style latent attention kernel for Trainium2, 8-core SPMD. v4.

Problem: B=4, S=2048, H=2048, NH=16, HD=64, KVC=512, causal softmax attention.

Sharding: core i handles batch b = i//2 and head-half hp = i%2 (8 heads).
Each core computes its partial c_proj output (contraction over its 512 of the
1024 attn-out dims); the host sums the two partials per batch.

v4 design (v2 was 401us, v3's pre-exp DVE mask regressed to 467us):
  - WAVEFRONT: only hidden-chunk 0 of the q/latent projections (phase A)
    and kT/v of chunk 0 (phase A2) run standalone.  Chunks 1-3 of A/A2 and
    all of c_proj are emitted as background PE work inside the attention
    tile loop (2 pops per tile), so the ACT-bound exp stream and the
    PE-bound projection work overlap instead of running serially.
  - The score->exp edge is kept engine-pure (PE->ACT): the causal mask is
    a post-exp multiply on the otherwise-idle GPSIMD engine (v2 had it on
    DVE where evictions convoyed ahead of it; v3's pre-exp DVE add was
    worse).  DVE keeps only PSUM work (evictions, epilogue).
  - v1 head slots are [1 | v(64)]: the AV matmul's denominator row lands
    on PSUM partition 0, so reciprocal_approx_fast reads it directly (its
    nonzero-base-partition bug avoided) - epilogue is recip + gpsimd
    broadcast + one DVE normalize-mul off the PSUM accumulator.
  - All SBUF operands bf16 (host pre-transposes hidden and packs
    partition-major: every DMA line is >=4KB contiguous); PSUM f32;
    output written as bf16 partials, host sums in f32.
"""

import os
import sys

import numpy as np

for _p in ("/opt/trn_rl_repo",):
    if os.path.isdir(_p) and _p not in sys.path:
        sys.path.append(_p)

import concourse.bass as bass  # noqa: E402
import concourse.mybir as mybir  # noqa: E402
from concourse import bacc, tile  # noqa: E402
from concourse.bass_utils import run_bass_kernel_spmd  # noqa: E402

F32 = mybir.dt.float32
BF16 = mybir.dt.bfloat16
NPBF16 = mybir.dt.np(BF16)

B, S, H = 4, 2048, 2048
NH, HD = 16, 64
KVC = 512
DL = 512          # local q/k/v dims per core (8 heads x 64)
NHL = 8           # local heads
P = 128
SCALE = 0.125

HT = H // P       # 16 h-tiles
DT = DL // P      # 4 d-tiles (also head-pair index g)
CT = KVC // P     # 4 c-tiles
ST = S // P       # 16 s-tiles
SC = 512          # hidden s-chunk
NCH = S // SC     # 4 chunks

_CACHE = {}


def build_program():
    """Build + compile the per-core Bass program. Returns the Bacc module."""
    nc = bacc.Bacc("TRN2", target_bir_lowering=False, debug=False,
                   num_devices=8)

    hsx = nc.dram_tensor("hsx", [P, NCH, HT, SC], BF16,
                         kind="ExternalInput").ap()
    wq = nc.dram_tensor("wq", [P, HT, DL], BF16, kind="ExternalInput").ap()
    wkv = nc.dram_tensor("wkv", [P, HT, KVC], BF16,
                         kind="ExternalInput").ap()
    wk = nc.dram_tensor("wk", [P, CT, DL], BF16, kind="ExternalInput").ap()
    wv = nc.dram_tensor("wv", [P, CT, DL], BF16, kind="ExternalInput").ap()
    wo = nc.dram_tensor("wo", [P, DT, H], BF16, kind="ExternalInput").ap()
    out = nc.dram_tensor("out", [ST, P, H], BF16, kind="ExternalOutput").ap()

    from contextlib import ExitStack

    with tile.TileContext(nc) as tc, ExitStack() as stack:
        consts = stack.enter_context(tc.tile_pool(name="consts", bufs=1))
        # bigmask[j, z] = 1.0 if z - j >= 384 else 0.  Multiplies the
        # 128-wide diagonal band of an exp'd score tile (causal mask).
        bigmask = consts.tile([P, 896], BF16)
        nc.gpsimd.memset(bigmask, 1.0)
        nc.gpsimd.affine_select(
            out=bigmask, in_=bigmask,
            compare_op=mybir.AluOpType.is_ge, fill=0.0,
            base=-384, pattern=[[1, 896]], channel_multiplier=-1,
        )
        # Touch exp once so the ACT table set loads during the prologue.
        dummy = consts.tile([1, 32], F32)
        nc.vector.memset(dummy, 0.0)
        nc.scalar.activation(dummy, dummy,
                             mybir.ActivationFunctionType.Exp, scale=SCALE)

        persistA = stack.enter_context(tc.tile_pool(name="persistA", bufs=1))
        qT = persistA.tile([P, DT, S], BF16, tag="qT")
        latT = persistA.tile([P, CT, S], BF16, tag="latT")
        kT = persistA.tile([P, DT, S], BF16, tag="kT")
        v1 = persistA.tile([P, ST, NHL * (HD + 1)], BF16, tag="v1")
        # OT aliases latT s-block ib (latT's reads all precede the writes).
        OT = [latT[:, :, ib * 512:(ib + 1) * 512] for ib in range(4)]

        wB = stack.enter_context(tc.tile_pool(name="wB", bufs=1))
        wk_sb = wB.tile([P, CT, DL], BF16, tag="wk")
        nc.gpsimd.dma_start(wk_sb, wk)
        wv_sb = wB.tile([P, CT, DL], BF16, tag="wv")
        nc.gpsimd.dma_start(wv_sb, wv)
        wo_sb = wB.tile([P, DT, H], BF16, tag="wo")
        nc.gpsimd.dma_start(wo_sb, wo)

        wA = stack.enter_context(tc.tile_pool(name="wA", bufs=1))
        hinp = stack.enter_context(tc.tile_pool(name="hin", bufs=2))
        wq_sb = wA.tile([P, HT, DL], BF16, tag="wq")
        nc.sync.dma_start(wq_sb, wq)
        wkv_sb = wA.tile([P, HT, KVC], BF16, tag="wkv")
        hin = {}
        hin[0] = hinp.tile([P, HT, SC], BF16, tag="hin", name="hin0")
        nc.sync.dma_start(hin[0], hsx[:, 0])
        nc.sync.dma_start(wkv_sb, wkv)
        hin[1] = hinp.tile([P, HT, SC], BF16, tag="hin", name="hin1")
        nc.sync.dma_start(hin[1], hsx[:, 1])

        def emit_qlat_chain(c, dt_, w_sb, dst, ps):
            """One [128,512] projection chain: 16 MMs + bf16 eviction.
            Returns list of emission thunks (4 MMs each + evict)."""
            items = []
            for hq in range(4):
                def mms(hq=hq, ps=ps, c=c, dt_=dt_, w_sb=w_sb):
                    for hi in range(4):
                        ht = hq * 4 + hi
                        nc.tensor.matmul(
                            ps, w_sb[:, ht, dt_ * P:(dt_ + 1) * P],
                            hin[c][:, ht, :],
                            start=(ht == 0), stop=(ht == HT - 1))
                items.append(mms)
            def evict(ps=ps, c=c, dt_=dt_, dst=dst):
                nc.vector.tensor_copy(
                    dst[:, dt_, c * SC:(c + 1) * SC], ps)
            items.append(evict)
            return items

        def emit_kt_chain(sc, dt_, ps):
            items = []
            def mms(sc=sc, dt_=dt_, ps=ps):
                for ct in range(CT):
                    nc.tensor.matmul(
                        ps, wk_sb[:, ct, dt_ * P:(dt_ + 1) * P],
                        latT[:, ct, sc * 512:(sc + 1) * 512],
                        start=(ct == 0), stop=(ct == CT - 1))
            items.append(mms)
            def evict(sc=sc, dt_=dt_, ps=ps):
                nc.vector.tensor_copy(
                    kT[:, dt_, sc * 512:(sc + 1) * 512], ps)
            items.append(evict)
            return items

        def emit_v_chain(st, ps):
            items = []
            def mms(st=st, ps=ps):
                for ct in range(CT):
                    nc.tensor.matmul(
                        ps, latT[:, ct, st * P:(st + 1) * P],
                        wv_sb[:, ct, :],
                        start=(ct == 0), stop=(ct == CT - 1))
            items.append(mms)
            def evict(st=st, ps=ps):
                nc.vector.tensor_copy(
                    v1[:, st, :].rearrange(
                        "p (h e) -> p h e", e=HD + 1)[:, :, :HD],
                    ps.rearrange("p (h e) -> p h e", e=HD))
            items.append(evict)
            return items

        # ---------------- phase A: all chunks of qT/latT ----------------
        with tc.tile_pool(name="psA", bufs=2, space="PSUM") as psA:
            for c in range(NCH):
                if c + 2 < NCH:
                    hin[c + 2] = hinp.tile([P, HT, SC], BF16, tag="hin",
                                           name=f"hin{c + 2}")
                    nc.sync.dma_start(hin[c + 2], hsx[:, c + 2])
                for dt_ in range(DT):
                    ps = psA.tile([P, SC], F32, tag="psq")
                    for it in emit_qlat_chain(c, dt_, wq_sb, qT, ps):
                        it()
                    ps2 = psA.tile([P, SC], F32, tag="pslat")
                    for it in emit_qlat_chain(c, dt_, wkv_sb, latT, ps2):
                        it()
        # ---------------- phase A2: kT, v1 ----------------
        with tc.tile_pool(name="onep", bufs=1) as onep, \
             tc.tile_pool(name="psA2", bufs=2, space="PSUM") as psA2:
            # ones column LAST in each head's 65-wide slot (partition base
            # of engine ops must be 32-aligned, so den lands at 64)
            ones_src = onep.tile([P, ST, NHL], F32, tag="ones_src")
            nc.vector.memset(ones_src, 1.0)
            nc.vector.tensor_copy(
                v1.rearrange("p s (h e) -> p s h e", e=HD + 1)[:, :, :, HD],
                ones_src)
            for sc in range(NCH):
                for dt_ in range(DT):
                    ps = psA2.tile([P, 512], F32, tag="psk")
                    for it in emit_kt_chain(sc, dt_, ps):
                        it()
                for s2 in range(4):
                    ps = psA2.tile([P, 512], F32, tag="psv")
                    for it in emit_v_chain(sc * 4 + s2, ps):
                        it()

        # ------- main phase: attention with background A/A2/c_proj -------
        def issue_scores(g, ib, jt, ps_sp):
            toff = (jt - 4 * ib) * P if jt >= 4 * ib else 0
            jts = slice(jt * P, (jt + 1) * P)
            iw = slice(ib * 512 + toff, (ib + 1) * 512)
            ps2 = ps_sp.tile([P, 2, 512], F32, tag="ps2")
            nc.tensor.matmul(
                ps2[:, 0, toff:], kT[0:HD, g, jts],
                qT[0:HD, g, iw], start=True, stop=True)
            nc.tensor.matmul(
                ps2[:, 1, toff:], kT[HD:P, g, jts],
                qT[HD:P, g, iw], start=True, stop=True)
            return ps2

        def attn_block(g, ib, ps_sp, ps_op, ptp, smallp,
                       first_ps2, next_block, bg_pop):
            se = (2 * g) * (HD + 1)
            so = (2 * g + 1) * (HD + 1)
            jt_max = 4 * (ib + 1)
            po_e = ps_op.tile([HD + 1, 512], F32, tag="po_e")
            po_o = ps_op.tile([HD + 1, 512], F32, tag="po_o")

            ps2_next = first_ps2 if first_ps2 is not None \
                else issue_scores(g, ib, 0, ps_sp)
            handoff = None
            for jt in range(jt_max):
                ps2 = ps2_next
                if jt + 1 < jt_max:
                    ps2_next = issue_scores(g, ib, jt + 1, ps_sp)
                elif next_block is not None:
                    handoff = issue_scores(*next_block, 0, ps_sp)
                bg_pop()
                toff = (jt - 4 * ib) * P if jt >= 4 * ib else 0
                pt2 = ptp.tile([P, 2, 512], BF16, tag="pt2")
                nc.scalar.activation(
                    pt2[:, :, toff:], ps2[:, :, toff:],
                    mybir.ActivationFunctionType.Exp, scale=SCALE)
                if jt >= 4 * ib:
                    # causal mask on GPSIMD: keeps the DVE eviction convoy
                    # off the exp->AV edge
                    mw = min(P, 512 - toff)
                    nc.gpsimd.tensor_mul(
                        out=pt2[:, :, toff:toff + mw],
                        in0=pt2[:, :, toff:toff + mw],
                        in1=bigmask[:, 384:384 + mw]
                        .rearrange("p (o f) -> p o f", o=1)
                        .broadcast_to((P, 2, mw)))
                last = jt == jt_max - 1
                nc.tensor.matmul(
                    po_e[:, toff:], v1[:, jt, se:se + HD + 1],
                    pt2[:, 0, toff:], start=(jt == 0), stop=last)
                nc.tensor.matmul(
                    po_o[:, toff:], v1[:, jt, so:so + HD + 1],
                    pt2[:, 1, toff:], start=(jt == 0), stop=last)
                bg_pop()
            for po, pbase in ((po_e, 0), (po_o, HD)):
                # po rows: [O^T(64) | den at 64].  Stage den at partition 0
                # (reciprocal_approx_fast mis-reads at nonzero base).
                dn = smallp.tile([1, 512], F32, tag="dn")
                nc.vector.tensor_copy(dn, po[HD:HD + 1, :])
                rc = smallp.tile([1, 512], F32, tag="rc")
                nc.vector.reciprocal_approx_fast(out=rc, in_=dn)
                rc64 = smallp.tile([HD, 512], F32, tag="rc64")
                nc.gpsimd.partition_broadcast(rc64, rc)
                nc.vector.tensor_mul(
                    out=OT[ib][pbase:pbase + HD, g, :],
                    in0=po[:HD, :], in1=rc64)
            return handoff

        def cproj_items(m, osbp, bgp):
            items = []
            osb = osbp.tile([P, H], BF16, tag="osb", name=f"osb{m}")
            ot = OT[m // 4]
            lm = m % 4
            for n in range(4):
                pc = bgp.tile([P, 512], F32, tag="bg", name=f"pc{m}_{n}")
                def mms(n=n, pc=pc, ot=ot, lm=lm):
                    for kt in range(DT):
                        nc.tensor.matmul(
                            pc, ot[:, kt, lm * P:(lm + 1) * P],
                            wo_sb[:, kt, n * 512:(n + 1) * 512],
                            start=(kt == 0), stop=(kt == DT - 1))
                items.append(mms)
                def evict(n=n, pc=pc, osb=osb, m=m):
                    nc.vector.tensor_copy(
                        osb[:, n * 512:(n + 1) * 512], pc)
                    if n == 3:
                        nc.sync.dma_start(out[m], osb)
                items.append(evict)
            return items

        with tc.tile_pool(name="ptp", bufs=6) as ptp, \
             tc.tile_pool(name="smallp", bufs=2) as smallp, \
             tc.tile_pool(name="osb_p", bufs=3) as osbp, \
             tc.tile_pool(name="ps_s", bufs=2, space="PSUM") as ps_sp, \
             tc.tile_pool(name="ps_o", bufs=1, space="PSUM") as ps_op, \
             tc.tile_pool(name="bgp", bufs=2, space="PSUM") as bgp:
            bg = []

            def bg_pop():
                if bg:
                    bg.pop(0)()

            def bg_flush():
                while bg:
                    bg.pop(0)()

            blocks = [(g, ib) for ib in range(4) for g in range(DT)]
            carry = None
            for k, (g, ib) in enumerate(blocks):
                if g == 0 and ib >= 1:
                    for m in range(4 * (ib - 1), 4 * ib):
                        bg.extend(cproj_items(m, osbp, bgp))
                nxt = blocks[k + 1] if k + 1 < len(blocks) else None
                carry = attn_block(g, ib, ps_sp, ps_op, ptp, smallp,
                                   carry, nxt, bg_pop)
                if g == DT - 1:
                    bg_flush()
            for m in range(12, ST):
                for it in cproj_items(m, osbp, bgp):
                    it()

    nc.compile()
    return nc


def _get_program():
    if "nc" not in _CACHE:
        _CACHE["nc"] = build_program()
    return _CACHE["nc"]


def make_in_maps(hidden_states, wq, w_kv_down, w_k_up, w_v_up, w_out):
    """Host-side sharding: core i -> (batch i//2, head-half i%2).

    All operands are converted to bf16 and packed partition-major so every
    DMA line is contiguous:
      hsx[p, c, ht, s] = hidden[b][c*512+s, ht*128+p]   (pre-transposed)
      w*[p, t, d] with the contraction dim split as (t, p)
    """
    hsx_b = []
    for b in range(B):
        hb = np.asarray(hidden_states[b], dtype=np.float32).astype(NPBF16)
        x = hb.T.reshape(HT, P, NCH, SC).transpose(1, 2, 0, 3).copy()
        hsx_b.append(x)
    wq = np.asarray(wq, dtype=np.float32)
    w_kv_down = np.asarray(w_kv_down, dtype=np.float32)
    w_k_up = np.asarray(w_k_up, dtype=np.float32)
    w_v_up = np.asarray(w_v_up, dtype=np.float32)
    w_out = np.asarray(w_out, dtype=np.float32)
    wkv_p = w_kv_down.astype(NPBF16).reshape(HT, P, KVC).transpose(1, 0, 2) \
        .copy()
    in_maps = []
    for i in range(8):
        b, hp = i // 2, i % 2
        sl = slice(hp * DL, (hp + 1) * DL)
        in_maps.append({
            "hsx": hsx_b[b],
            "wq": wq[:, sl].astype(NPBF16).reshape(HT, P, DL)
            .transpose(1, 0, 2).copy(),
            "wkv": wkv_p,
            "wk": w_k_up[:, sl].astype(NPBF16).reshape(CT, P, DL)
            .transpose(1, 0, 2).copy(),
            "wv": w_v_up[:, sl].astype(NPBF16).reshape(CT, P, DL)
            .transpose(1, 0, 2).copy(),
            "wo": w_out[sl, :].astype(NPBF16).reshape(DT, P, H)
            .transpose(1, 0, 2).copy(),
        })
    return in_maps


def gather(results):
    """Host-side unshard: sum the two head-half bf16 partials per batch."""
    out = np.empty((B, S, H), dtype=np.float32)
    for b in range(B):
        p0 = np.asarray(results[2 * b]["out"]).astype(np.float32)
        p1 = np.asarray(results[2 * b + 1]["out"]).astype(np.float32)
        out[b] = (p0 + p1).reshape(S, H)
    return out


def kernel(hidden_states, wq, w_kv_down, w_k_up, w_v_up, w_out, _trace=False):
    nc = _get_program()
    in_maps = make_in_maps(hidden_states, wq, w_kv_down, w_k_up, w_v_up,
                           w_out)
    res = run_bass_kernel_spmd(nc, in_maps, list(range(8)), trace=_trace)
    out = gather(res.results)
    if _trace:
        return out, res
    return out


# revision 19
# speedup vs baseline: 1.4393x; 1.4083x over previous
"""MLA-style latent attention kernel for Trainium2, 8-core SPMD. v4.

Problem: B=4, S=2048, H=2048, NH=16, HD=64, KVC=512, causal softmax attention.

Sharding: core i handles batch b = i//2 and head-half hp = i%2 (8 heads).
Each core computes its partial c_proj output (contraction over its 512 of the
1024 attn-out dims); the host sums the two partials per batch.

v4 design (v2 was 401us, v3's pre-exp DVE mask regressed to 467us):
  - WAVEFRONT: only hidden-chunk 0 of the q/latent projections (phase A)
    and kT/v of chunk 0 (phase A2) run standalone.  Chunks 1-3 of A/A2 and
    all of c_proj are emitted as background PE work inside the attention
    tile loop (2 pops per tile), so the ACT-bound exp stream and the
    PE-bound projection work overlap instead of running serially.
  - The score->exp edge is kept engine-pure (PE->ACT): the causal mask is
    a post-exp multiply on the otherwise-idle GPSIMD engine (v2 had it on
    DVE where evictions convoyed ahead of it; v3's pre-exp DVE add was
    worse).  DVE keeps only PSUM work (evictions, epilogue).
  - v1 head slots are [1 | v(64)]: the AV matmul's denominator row lands
    on PSUM partition 0, so reciprocal_approx_fast reads it directly (its
    nonzero-base-partition bug avoided) - epilogue is recip + gpsimd
    broadcast + one DVE normalize-mul off the PSUM accumulator.
  - All SBUF operands bf16 (host pre-transposes hidden and packs
    partition-major: every DMA line is >=4KB contiguous); PSUM f32;
    output written as bf16 partials, host sums in f32.
"""

import os
import sys

import numpy as np

for _p in ("/opt/trn_rl_repo",):
    if os.path.isdir(_p) and _p not in sys.path:
        sys.path.append(_p)

import concourse.bass as bass  # noqa: E402
import concourse.mybir as mybir  # noqa: E402
from concourse import bacc, tile  # noqa: E402
from concourse.bass_utils import run_bass_kernel_spmd  # noqa: E402

F32 = mybir.dt.float32
BF16 = mybir.dt.bfloat16
NPBF16 = mybir.dt.np(BF16)

B, S, H = 4, 2048, 2048
NH, HD = 16, 64
KVC = 512
DL = 512          # local q/k/v dims per core (8 heads x 64)
NHL = 8           # local heads
P = 128
SCALE = 0.125

HT = H // P       # 16 h-tiles
DT = DL // P      # 4 d-tiles (also head-pair index g)
CT = KVC // P     # 4 c-tiles
ST = S // P       # 16 s-tiles
SC = 512          # hidden s-chunk
NCH = S // SC     # 4 chunks

_CACHE = {}


def build_program():
    """Build + compile the per-core Bass program. Returns the Bacc module."""
    nc = bacc.Bacc("TRN2", target_bir_lowering=False, debug=False,
                   num_devices=8)

    hsx = nc.dram_tensor("hsx", [P, NCH, HT, SC], BF16,
                         kind="ExternalInput").ap()
    wq = nc.dram_tensor("wq", [P, HT, DL], BF16, kind="ExternalInput").ap()
    wkv = nc.dram_tensor("wkv", [P, HT, KVC], BF16,
                         kind="ExternalInput").ap()
    wk = nc.dram_tensor("wk", [P, CT, DL], BF16, kind="ExternalInput").ap()
    wv = nc.dram_tensor("wv", [P, CT, DL], BF16, kind="ExternalInput").ap()
    wo = nc.dram_tensor("wo", [P, DT, H], BF16, kind="ExternalInput").ap()
    out = nc.dram_tensor("out", [ST, P, H], BF16, kind="ExternalOutput").ap()

    from contextlib import ExitStack

    with tile.TileContext(nc) as tc, ExitStack() as stack:
        consts = stack.enter_context(tc.tile_pool(name="consts", bufs=1))
        # bigmask[j, z] = 1.0 if z - j >= 384 else 0.  Multiplies the
        # 128-wide diagonal band of an exp'd score tile (causal mask).
        bigmask = consts.tile([P, 896], BF16)
        nc.gpsimd.memset(bigmask, 1.0)
        nc.gpsimd.affine_select(
            out=bigmask, in_=bigmask,
            compare_op=mybir.AluOpType.is_ge, fill=0.0,
            base=-384, pattern=[[1, 896]], channel_multiplier=-1,
        )
        # Touch exp once so the ACT table set loads during the prologue.
        dummy = consts.tile([1, 32], F32)
        nc.vector.memset(dummy, 0.0)
        nc.scalar.activation(dummy, dummy,
                             mybir.ActivationFunctionType.Exp, scale=SCALE)

        persistA = stack.enter_context(tc.tile_pool(name="persistA", bufs=1))
        qT = persistA.tile([P, DT, S], BF16, tag="qT")
        latT = persistA.tile([P, CT, S], BF16, tag="latT")
        kT = persistA.tile([P, DT, S], BF16, tag="kT")
        v1 = persistA.tile([P, ST, NHL * (HD + 1)], BF16, tag="v1")
        # One OT tile per (query-block, head-pair): c_proj matmuls of block
        # ib-1 never pick up false deps on block ib's in-flight epilogues.
        OT = [[persistA.tile([P, 512], BF16, tag=f"OT{ib}_{g}",
                             name=f"OT{ib}_{g}")
               for g in range(DT)] for ib in range(4)]

        wB = stack.enter_context(tc.tile_pool(name="wB", bufs=1))
        wk_sb = wB.tile([P, CT, DL], BF16, tag="wk")
        nc.gpsimd.dma_start(wk_sb, wk)
        wv_sb = wB.tile([P, CT, DL], BF16, tag="wv")
        nc.gpsimd.dma_start(wv_sb, wv)
        wo_sb = wB.tile([P, DT, H], BF16, tag="wo")
        nc.gpsimd.dma_start(wo_sb, wo)

        wA_ctx = tc.tile_pool(name="wA", bufs=1)
        hin_ctx = tc.tile_pool(name="hin", bufs=2)
        wA = wA_ctx.__enter__()
        hinp = hin_ctx.__enter__()
        wq_sb = wA.tile([P, HT, DL], BF16, tag="wq")
        nc.sync.dma_start(wq_sb, wq)
        wkv_sb = wA.tile([P, HT, KVC], BF16, tag="wkv")
        hin = {}
        hin[0] = hinp.tile([P, HT, SC], BF16, tag="hin", name="hin0")
        nc.sync.dma_start(hin[0], hsx[:, 0])
        nc.sync.dma_start(wkv_sb, wkv)
        hin[1] = hinp.tile([P, HT, SC], BF16, tag="hin", name="hin1")
        nc.sync.dma_start(hin[1], hsx[:, 1])

        def emit_qlat_chain(c, dt_, w_sb, dst, ps):
            """One [128,512] projection chain: 16 MMs + bf16 eviction.
            Returns list of emission thunks (4 MMs each + evict)."""
            items = []
            for hq in range(4):
                def mms(hq=hq, ps=ps, c=c, dt_=dt_, w_sb=w_sb):
                    for hi in range(4):
                        ht = hq * 4 + hi
                        nc.tensor.matmul(
                            ps, w_sb[:, ht, dt_ * P:(dt_ + 1) * P],
                            hin[c][:, ht, :],
                            start=(ht == 0), stop=(ht == HT - 1))
                items.append(mms)
            def evict(ps=ps, c=c, dt_=dt_, dst=dst):
                nc.vector.tensor_copy(
                    dst[:, dt_, c * SC:(c + 1) * SC], ps)
            items.append(evict)
            return items

        def emit_kt_chain(sc, dt_, ps):
            items = []
            def mms(sc=sc, dt_=dt_, ps=ps):
                for ct in range(CT):
                    nc.tensor.matmul(
                        ps, wk_sb[:, ct, dt_ * P:(dt_ + 1) * P],
                        latT[:, ct, sc * 512:(sc + 1) * 512],
                        start=(ct == 0), stop=(ct == CT - 1))
            items.append(mms)
            def evict(sc=sc, dt_=dt_, ps=ps):
                nc.vector.tensor_copy(
                    kT[:, dt_, sc * 512:(sc + 1) * 512], ps)
            items.append(evict)
            return items

        def emit_v_chain(st, ps):
            items = []
            def mms(st=st, ps=ps):
                for ct in range(CT):
                    nc.tensor.matmul(
                        ps, latT[:, ct, st * P:(st + 1) * P],
                        wv_sb[:, ct, :],
                        start=(ct == 0), stop=(ct == CT - 1))
            items.append(mms)
            def evict(st=st, ps=ps):
                nc.vector.tensor_copy(
                    v1[:, st, :].rearrange(
                        "p (h e) -> p h e", e=HD + 1)[:, :, :HD],
                    ps.rearrange("p (h e) -> p h e", e=HD))
            items.append(evict)
            return items

        # ---------------- phase A: all chunks of qT/latT ----------------
        with tc.tile_pool(name="psA", bufs=2, space="PSUM") as psA:
            for c in range(NCH):
                if c + 2 < NCH:
                    hin[c + 2] = hinp.tile([P, HT, SC], BF16, tag="hin",
                                           name=f"hin{c + 2}")
                    nc.sync.dma_start(hin[c + 2], hsx[:, c + 2])
                for dt_ in range(DT):
                    ps = psA.tile([P, SC], F32, tag="psq")
                    for it in emit_qlat_chain(c, dt_, wq_sb, qT, ps):
                        it()
                    ps2 = psA.tile([P, SC], F32, tag="pslat")
                    for it in emit_qlat_chain(c, dt_, wkv_sb, latT, ps2):
                        it()
        hin_ctx.__exit__(None, None, None)
        wA_ctx.__exit__(None, None, None)
        # ---------------- phase A2: kT, v1 ----------------
        with tc.tile_pool(name="onep", bufs=1) as onep, \
             tc.tile_pool(name="psA2", bufs=2, space="PSUM") as psA2:
            # ones column LAST in each head's 65-wide slot (partition base
            # of engine ops must be 32-aligned, so den lands at 64)
            ones_src = onep.tile([P, ST, NHL], F32, tag="ones_src")
            nc.vector.memset(ones_src, 1.0)
            nc.vector.tensor_copy(
                v1.rearrange("p s (h e) -> p s h e", e=HD + 1)[:, :, :, HD],
                ones_src)
            for sc in range(NCH):
                for dt_ in range(DT):
                    ps = psA2.tile([P, 512], F32, tag="psk")
                    for it in emit_kt_chain(sc, dt_, ps):
                        it()
                for s2 in range(4):
                    ps = psA2.tile([P, 512], F32, tag="psv")
                    for it in emit_v_chain(sc * 4 + s2, ps):
                        it()

        # ------- main phase: attention with background A/A2/c_proj -------
        def issue_scores(g, ib, jt, ps_sp):
            toff = (jt - 4 * ib) * P if jt >= 4 * ib else 0
            jts = slice(jt * P, (jt + 1) * P)
            iw = slice(ib * 512 + toff, (ib + 1) * 512)
            ps2 = ps_sp.tile([P, 2, 512], F32, tag="ps2")
            nc.tensor.matmul(
                ps2[:, 0, toff:], kT[0:HD, g, jts],
                qT[0:HD, g, iw], start=True, stop=True)
            nc.tensor.matmul(
                ps2[:, 1, toff:], kT[HD:P, g, jts],
                qT[HD:P, g, iw], start=True, stop=True)
            return ps2

        def attn_block(g, ib, ps_sp, ps_op, ptp, smallp,
                       first_ps2, next_block, bg_pop):
            se = (2 * g) * (HD + 1)
            so = (2 * g + 1) * (HD + 1)
            jt_max = 4 * (ib + 1)
            po_e = ps_op.tile([HD + 1, 512], F32, tag="po_e")
            po_o = ps_op.tile([HD + 1, 512], F32, tag="po_o")

            ps2_next = first_ps2 if first_ps2 is not None \
                else issue_scores(g, ib, 0, ps_sp)
            handoff = None
            for jt in range(jt_max):
                ps2 = ps2_next
                if jt + 1 < jt_max:
                    ps2_next = issue_scores(g, ib, jt + 1, ps_sp)
                elif next_block is not None:
                    handoff = issue_scores(*next_block, 0, ps_sp)
                bg_pop()
                toff = (jt - 4 * ib) * P if jt >= 4 * ib else 0
                pt2 = ptp.tile([P, 2, 512], BF16, tag="pt2")
                nc.scalar.activation(
                    pt2[:, :, toff:], ps2[:, :, toff:],
                    mybir.ActivationFunctionType.Exp, scale=SCALE)
                if jt >= 4 * ib:
                    # causal mask on the 128-wide diagonal band (DVE;
                    # GPSIMD's ~1.7us/op issue overhead poisons this edge)
                    mw = min(P, 512 - toff)
                    nc.vector.tensor_mul(
                        out=pt2[:, :, toff:toff + mw],
                        in0=pt2[:, :, toff:toff + mw],
                        in1=bigmask[:, 384:384 + mw]
                        .rearrange("p (o f) -> p o f", o=1)
                        .broadcast_to((P, 2, mw)))
                last = jt == jt_max - 1
                nc.tensor.matmul(
                    po_e[:, toff:], v1[:, jt, se:se + HD + 1],
                    pt2[:, 0, toff:], start=(jt == 0), stop=last)
                nc.tensor.matmul(
                    po_o[:, toff:], v1[:, jt, so:so + HD + 1],
                    pt2[:, 1, toff:], start=(jt == 0), stop=last)
                bg_pop()
            for po, pbase in ((po_e, 0), (po_o, HD)):
                # po rows: [O^T(64) | den at 64].  Stage den at partition 0
                # (reciprocal_approx_fast mis-reads at nonzero base).
                dn = smallp.tile([1, 512], F32, tag="dn")
                nc.vector.tensor_copy(dn, po[HD:HD + 1, :])
                rc = smallp.tile([1, 512], F32, tag="rc")
                nc.vector.reciprocal_approx_fast(out=rc, in_=dn)
                rc64 = smallp.tile([HD, 512], F32, tag="rc64")
                nc.gpsimd.partition_broadcast(rc64, rc)
                nc.vector.tensor_mul(
                    out=OT[ib][g][pbase:pbase + HD, :],
                    in0=po[:HD, :], in1=rc64)
            return handoff

        def cproj_items(m, osbp, bgp):
            items = []
            osb = osbp.tile([P, H], BF16, tag="osb", name=f"osb{m}")
            ot = OT[m // 4]
            lm = m % 4
            for n in range(4):
                pc = bgp.tile([P, 512], F32, tag="bg", name=f"pc{m}_{n}")
                def mms(n=n, pc=pc, ot=ot, lm=lm):
                    for kt in range(DT):
                        nc.tensor.matmul(
                            pc, ot[kt][:, lm * P:(lm + 1) * P],
                            wo_sb[:, kt, n * 512:(n + 1) * 512],
                            start=(kt == 0), stop=(kt == DT - 1))
                items.append(mms)
                def evict(n=n, pc=pc, osb=osb, m=m):
                    nc.vector.tensor_copy(
                        osb[:, n * 512:(n + 1) * 512], pc)
                    if n == 3:
                        nc.sync.dma_start(out[m], osb)
                items.append(evict)
            return items

        with tc.tile_pool(name="ptp", bufs=6) as ptp, \
             tc.tile_pool(name="smallp", bufs=2) as smallp, \
             tc.tile_pool(name="osb_p", bufs=3) as osbp, \
             tc.tile_pool(name="ps_s", bufs=2, space="PSUM") as ps_sp, \
             tc.tile_pool(name="ps_o", bufs=1, space="PSUM") as ps_op, \
             tc.tile_pool(name="bgp", bufs=2, space="PSUM") as bgp:
            bg = []

            def bg_pop():
                if bg:
                    bg.pop(0)()

            def bg_flush():
                while bg:
                    bg.pop(0)()

            blocks = [(g, ib) for ib in range(4) for g in range(DT)]
            carry = None
            for k, (g, ib) in enumerate(blocks):
                if g == 0 and ib >= 1:
                    for m in range(4 * (ib - 1), 4 * ib):
                        bg.extend(cproj_items(m, osbp, bgp))
                nxt = blocks[k + 1] if k + 1 < len(blocks) else None
                carry = attn_block(g, ib, ps_sp, ps_op, ptp, smallp,
                                   carry, nxt, bg_pop)
                if g == DT - 1:
                    bg_flush()
            for m in range(12, ST):
                for it in cproj_items(m, osbp, bgp):
                    it()

    nc.compile()
    return nc


def _get_program():
    if "nc" not in _CACHE:
        _CACHE["nc"] = build_program()
    return _CACHE["nc"]


def make_in_maps(hidden_states, wq, w_kv_down, w_k_up, w_v_up, w_out):
    """Host-side sharding: core i -> (batch i//2, head-half i%2).

    All operands are converted to bf16 and packed partition-major so every
    DMA line is contiguous:
      hsx[p, c, ht, s] = hidden[b][c*512+s, ht*128+p]   (pre-transposed)
      w*[p, t, d] with the contraction dim split as (t, p)
    """
    hsx_b = []
    for b in range(B):
        hb = np.asarray(hidden_states[b], dtype=np.float32).astype(NPBF16)
        x = hb.T.reshape(HT, P, NCH, SC).transpose(1, 2, 0, 3).copy()
        hsx_b.append(x)
    wq = np.asarray(wq, dtype=np.float32)
    w_kv_down = np.asarray(w_kv_down, dtype=np.float32)
    w_k_up = np.asarray(w_k_up, dtype=np.float32)
    w_v_up = np.asarray(w_v_up, dtype=np.float32)
    w_out = np.asarray(w_out, dtype=np.float32)
    wkv_p = w_kv_down.astype(NPBF16).reshape(HT, P, KVC).transpose(1, 0, 2) \
        .copy()
    in_maps = []
    for i in range(8):
        b, hp = i // 2, i % 2
        sl = slice(hp * DL, (hp + 1) * DL)
        in_maps.append({
            "hsx": hsx_b[b],
            "wq": wq[:, sl].astype(NPBF16).reshape(HT, P, DL)
            .transpose(1, 0, 2).copy(),
            "wkv": wkv_p,
            "wk": w_k_up[:, sl].astype(NPBF16).reshape(CT, P, DL)
            .transpose(1, 0, 2).copy(),
            "wv": w_v_up[:, sl].astype(NPBF16).reshape(CT, P, DL)
            .transpose(1, 0, 2).copy(),
            "wo": w_out[sl, :].astype(NPBF16).reshape(DT, P, H)
            .transpose(1, 0, 2).copy(),
        })
    return in_maps


def gather(results):
    """Host-side unshard: sum the two head-half bf16 partials per batch."""
    out = np.empty((B, S, H), dtype=np.float32)
    for b in range(B):
        p0 = np.asarray(results[2 * b]["out"]).astype(np.float32)
        p1 = np.asarray(results[2 * b + 1]["out"]).astype(np.float32)
        out[b] = (p0 + p1).reshape(S, H)
    return out


def kernel(hidden_states, wq, w_kv_down, w_k_up, w_v_up, w_out, _trace=False):
    nc = _get_program()
    in_maps = make_in_maps(hidden_states, wq, w_kv_down, w_k_up, w_v_up,
                           w_out)
    res = run_bass_kernel_spmd(nc, in_maps, list(range(8)), trace=_trace)
    out = gather(res.results)
    if _trace:
        return out, res
    return out
